# revision 4
# baseline (speedup 1.0000x reference)
"""Trainium2 Bass kernel for CombineAttention (B=2, T=4096, sT=1024, C=1024, H=16, D=64).

Sharding: 8 cores = 2 batches x 4 head-groups (4 heads each).
Host pre-transposes activations/weights so every on-device matmul has its
contraction dim on partitions; the monotonic mask (query i attends keys
<= 4i+3) becomes a block-causal structure handled by suffix-restricted
matmuls plus one small static (128,32) diagonal-band mask.

Precision: projections/scores run in float32r (full PE rate at free-dim
>= 256, ~tf32 accuracy); the attention-weights path (v, exp(scores))
runs in bf16, which contributes <0.5% relative error. PSUM accumulation
is fp32 throughout; the softmax needs no max-subtraction (|score| < ~55
fits fp32 exp comfortably), and a ones-column appended to v makes the
softmax normalizer fall out of the attn@v matmul for free.

Per-core pipeline:
  qsT = WqT.T @ sxT          (256,1024)   q-scale folded into WqT on host
  kT  = WkT.T @ xT           (256,4096)   streamed in 512-key slices
  v   = xT.T  @ WvT          (4096,256) + ones column
  per head: scoresT = kT_tile.T @ qsT ; exp ; mask band ; yT_aug = v_aug.T @ expT
  normalize rows by the ones-column sum; out_partial = yT.T_chunks @ WcT
Host sums the 4 head-group partials per batch.
"""

import math
from contextlib import ExitStack

import numpy as np
import ml_dtypes

import concourse.bass as bass
import concourse.tile as tile
from concourse import bacc, mybir
from concourse.bass import ts, ds

BF16 = mybir.dt.bfloat16
FP32 = mybir.dt.float32
FP32R = mybir.dt.float32r

B = 2
C = 1024
T = 4096
ST = 1024
H = 16
D = 64
HO = 256          # head-group output channels per core (4 heads)
NCC = C // 128    # 8 contraction chunks
NTT = T // 128    # 32 key tiles
NKC = T // 512    # 8 key slices (projection streaming)
NQC = ST // 512   # 2 query chunks (attention)
NQT = ST // 128   # 8 query tiles (c-projection)


def build_nc(masked: bool = True):
    nc = bacc.Bacc("TRN2", target_bir_lowering=False, debug=False, num_devices=8)
    xT = nc.dram_tensor("xT", [C, T], FP32, kind="ExternalInput").ap()
    sxT = nc.dram_tensor("sxT", [C, ST], FP32, kind="ExternalInput").ap()
    wq = nc.dram_tensor("wq", [C, HO], FP32, kind="ExternalInput").ap()
    wk = nc.dram_tensor("wk", [C, HO], FP32, kind="ExternalInput").ap()
    wv = nc.dram_tensor("wv", [C, HO], FP32, kind="ExternalInput").ap()
    wc = nc.dram_tensor("wc", [HO, C], FP32, kind="ExternalInput").ap()
    maskd = nc.dram_tensor("mask", [128, 32], BF16, kind="ExternalInput").ap()
    out = nc.dram_tensor("out", [ST, C], FP32, kind="ExternalOutput").ap()

    r32 = lambda ap: ap.bitcast(FP32R)

    with tile.TileContext(nc) as tc, ExitStack() as ctx:
        const = ctx.enter_context(tc.tile_pool(name="const", bufs=1))
        big = ctx.enter_context(tc.tile_pool(name="big", bufs=1))
        xsl_pool = ctx.enter_context(tc.tile_pool(name="xsl", bufs=3))
        work = ctx.enter_context(tc.tile_pool(name="work", bufs=6))
        nrm = ctx.enter_context(tc.tile_pool(name="nrm", bufs=3))
        outw = ctx.enter_context(tc.tile_pool(name="outw", bufs=3))

        wq_sb = const.tile([128, NCC, HO], FP32R, tag="wq")
        wk_sb = const.tile([128, NCC, HO], FP32R, tag="wk")
        wv_sb = const.tile([128, NCC, HO], FP32R, tag="wv")
        wc_sb = const.tile([128, 2, C], FP32R, tag="wc")
        mask_sb = const.tile([128, 32], BF16, tag="mask")

        kT_sb = big.tile([128, 2, T], FP32R, tag="kT")
        qsT_sb = big.tile([128, 2, ST], FP32R, tag="qsT")
        v_sb = big.tile([128, NTT, 4, 65], BF16, tag="v")
        yT_sb = big.tile([128, 2, ST], FP32R, tag="yT")

        # ---- weight DMAs ----
        nc.sync.dma_start(wq_sb[:], r32(wq.rearrange("(cc p) o -> p cc o", p=128)))
        nc.sync.dma_start(wk_sb[:], r32(wk.rearrange("(cc p) o -> p cc o", p=128)))
        nc.sync.dma_start(wv_sb[:], r32(wv.rearrange("(cc p) o -> p cc o", p=128)))
        for kk in range(2):
            nc.sync.dma_start(wc_sb[:, kk, :], r32(wc[ts(kk, 128), :]))
        nc.sync.dma_start(mask_sb[:], maskd[:])

        with tc.tile_pool(name="psA", bufs=2, space="PSUM") as pp, \
             tc.tile_pool(name="psS", bufs=4, space="PSUM") as scp, \
             tc.tile_pool(name="psV", bufs=2, space="PSUM") as avp:

            # ---- q projection: qsT (256, ST) ----
            for qc in range(NQC):
                sxsl = xsl_pool.tile([128, NCC, 512], FP32R, tag="xsl", name=f"sxsl{qc}")
                for cc in range(NCC):
                    nc.sync.dma_start(sxsl[:, cc, :], r32(sxT[ts(cc, 128), ts(qc, 512)]))
                for ot in range(2):
                    ps = pp.tile([128, 512], FP32, tag="proj")
                    for cc in range(NCC):
                        nc.tensor.matmul(
                            ps[:],
                            wq_sb[:, cc, ts(ot, 128)],
                            sxsl[:, cc, :],
                            start=(cc == 0),
                            stop=(cc == NCC - 1),
                        )
                    nc.vector.tensor_copy(qsT_sb[:, ot, ts(qc, 512)], ps[:])

            def proj_slice(kc):
                """k/v projections for key slice [512*kc, 512*kc+512)."""
                xsl = xsl_pool.tile([128, NCC, 512], FP32R, tag="xsl", name=f"xsl{kc}")
                for cc in range(NCC):
                    nc.sync.dma_start(xsl[:, cc, :], r32(xT[ts(cc, 128), ts(kc, 512)]))
                for ot in range(2):
                    ps = pp.tile([128, 512], FP32, tag="proj")
                    for cc in range(NCC):
                        nc.tensor.matmul(
                            ps[:],
                            wk_sb[:, cc, ts(ot, 128)],
                            xsl[:, cc, :],
                            start=(cc == 0),
                            stop=(cc == NCC - 1),
                        )
                    nc.vector.tensor_copy(kT_sb[:, ot, ts(kc, 512)], ps[:])
                for tl in range(4):
                    tt = 4 * kc + tl
                    ps = pp.tile([128, 512], FP32, tag="proj")
                    pv = ps[:, 0:256]
                    for cc in range(NCC):
                        nc.tensor.matmul(
                            pv,
                            xsl[:, cc, ts(tl, 128)],
                            wv_sb[:, cc, :],
                            start=(cc == 0),
                            stop=(cc == NCC - 1),
                        )
                    nc.vector.tensor_copy(
                        v_sb[:, tt, :, 0:64], pv.rearrange("p (h d) -> p h d", h=4)
                    )
                    nc.vector.memset(v_sb[:, tt, :, 64:65], 1.0)

            def attn_unit(ot, qc):
                """Attention for heads (2*ot, 2*ot+1), queries [512*qc, 512*qc+512)."""
                ntiles = 16 * (qc + 1) if masked else NTT
                avps = [
                    avp.tile([65, 512], FP32, tag="av", name=f"av{ot}{qc}{hh}")
                    for hh in range(2)
                ]
                for tt in range(ntiles):
                    r = tt - 16 * qc if masked else -1  # >= 0: diagonal-band tile
                    col0 = 32 * r if r >= 0 else 0
                    width = 512 - col0
                    # fp32r matmul is full-rate only at N >= 256: widen the
                    # window backward over real (lower) query columns; exp
                    # then reads only the true suffix.
                    ext = max(0, 256 - width)
                    cw = width + ext
                    for h in range(2):
                        row = ds(64 * h, 64)
                        sc = scp.tile([128, 512], FP32, tag="sc")
                        nc.tensor.matmul(
                            sc[:, 0:cw],
                            kT_sb[row, ot, ts(tt, 128)],
                            qsT_sb[row, ot, ds(512 * qc + col0 - ext, cw)],
                            start=True,
                            stop=True,
                        )
                        ex = work.tile([128, 512], BF16, tag="exp")
                        nc.scalar.activation(
                            ex[:, 0:width],
                            sc[:, ds(ext, width)],
                            mybir.ActivationFunctionType.Exp,
                        )
                        if r >= 0:
                            nc.vector.tensor_mul(ex[:, 0:32], ex[:, 0:32], mask_sb[:])
                        nc.tensor.matmul(
                            avps[h][:, ds(col0, width)],
                            v_sb[:, tt, 2 * ot + h, :],
                            ex[:, 0:width],
                            start=(tt == 0),
                            stop=(tt == ntiles - 1),
                        )
                # normalize: y = yT_unnorm / l  (l = ones-column row of av)
                for h in range(2):
                    linv = nrm.tile([1, 512], FP32, tag="linv")
                    nc.vector.reciprocal(linv[:], avps[h][64:65, :])
                    bc = nrm.tile([64, 512], FP32, tag="bc")
                    nc.sync.dma_start(
                        bc[:], linv[:].unsqueeze(1).broadcast_to([1, 64, 512])
                    )
                    nc.vector.tensor_mul(
                        yT_sb[ds(64 * h, 64), ot, ts(qc, 512)],
                        avps[h][0:64, :],
                        bc[:],
                    )

            if masked:
                for kc in range(4):
                    proj_slice(kc)
                attn_unit(0, 0)
                attn_unit(1, 0)
                for kc in range(4, NKC):
                    proj_slice(kc)
                attn_unit(0, 1)
                attn_unit(1, 1)
            else:
                for kc in range(NKC):
                    proj_slice(kc)
                for qc in range(NQC):
                    for ot in range(2):
                        attn_unit(ot, qc)

        # ---- c projection: out_partial (ST, C) ----
        with tc.tile_pool(name="psO", bufs=2, space="PSUM") as outp:
            for nt in range(NQT):
                for ec in range(2):
                    ps = outp.tile([128, 512], FP32, tag="out")
                    for kk in range(2):
                        nc.tensor.matmul(
                            ps[:],
                            yT_sb[:, kk, ts(nt, 128)],
                            wc_sb[:, kk, ts(ec, 512)],
                            start=(kk == 0),
                            stop=(kk == 1),
                        )
                    osb = outw.tile([128, 512], FP32, tag="osb")
                    nc.vector.tensor_copy(osb[:], ps[:])
                    nc.sync.dma_start(out[ts(nt, 128), ts(ec, 512)], osb[:])

    nc.compile()
    return nc


_NC_CACHE = {}


def _get_nc(masked: bool):
    if masked not in _NC_CACHE:
        _NC_CACHE[masked] = build_nc(masked)
    return _NC_CACHE[masked]


def _shard_inputs(x, sx, Wq, Wk, Wv, Wc, qm):
    bf = ml_dtypes.bfloat16
    t_len = x.shape[1]
    qscale = math.log(t_len) / math.sqrt(D)
    qmfull = np.tile(np.asarray(qm, np.float32), 4) * qscale  # (256,)

    tk = np.arange(128)[:, None]
    cl = np.arange(32)[None, :]
    mask = (cl >= tk // 4).astype(np.float32).astype(bf)

    in_maps = []
    for b in range(B):
        xT = np.ascontiguousarray(x[b].T)
        sxT = np.ascontiguousarray(sx[b].T)
        for hg in range(4):
            sl = slice(hg * HO, (hg + 1) * HO)
            in_maps.append(
                {
                    "xT": xT,
                    "sxT": sxT,
                    "wq": np.ascontiguousarray((Wq[sl, :] * qmfull[:, None]).T),
                    "wk": np.ascontiguousarray(Wk[sl, :].T),
                    "wv": np.ascontiguousarray(Wv[sl, :].T),
                    "wc": np.ascontiguousarray(Wc[:, sl].T),
                    "mask": mask,
                }
            )
    return in_maps


def _run(inputs, trace=False):
    from concourse.bass_utils import run_bass_kernel_spmd

    x = np.asarray(inputs["x"], np.float32)
    sx = np.asarray(inputs["sx"], np.float32)
    Wq = np.asarray(inputs["Wq"], np.float32)
    Wk = np.asarray(inputs["Wk"], np.float32)
    Wv = np.asarray(inputs["Wv"], np.float32)
    Wc = np.asarray(inputs["Wc"], np.float32)
    qm = np.asarray(inputs["qm"], np.float32)
    causal = int(np.asarray(inputs.get("causal", 1)))
    masked = bool(causal) and sx.shape[1] != x.shape[1]

    nc = _get_nc(masked)
    in_maps = _shard_inputs(x, sx, Wq, Wk, Wv, Wc, qm)
    kwargs = {}
    if trace:
        kwargs = dict(trace=True, trace_cores=list(range(8)))
    res = run_bass_kernel_spmd(nc, in_maps, core_ids=list(range(8)), **kwargs)

    out = np.zeros((B, ST, C), np.float32)
    for b in range(B):
        for hg in range(4):
            out[b] += res.results[b * 4 + hg]["out"]
    return out, res


def kernel(**inputs):
    out, _ = _run(inputs, trace=False)
    return out


def kernel_traced(**inputs):
    out, res = _run(inputs, trace=True)
    return out, res
